# revision 18
# baseline (speedup 1.0000x reference)
"""Trainium2 Bass kernel for EnhancedMessageLayer (GNN message passing).

Strategy (8 NeuronCores, no collectives):
  * Nodes are split into 8 contiguous slices of 6250; every edge is owned by
    the core that owns its dst node.  Each core computes the full layer for
    its node slice; host concatenates.
  * Per core, nodes are processed in 49 tiles of 128 nodes (last tile
    overlaps; host discards the duplicated rows).  Edges are bucketed by dst
    tile and padded to chunks of 128.
  * Per tile: one bf16 transposed dma_gather fetches x[src].T for all the
    tile's edges ([h, e] layout, ready as matmul stationary).  The dst
    one-hot indicator is built once per tile in the [n, e] orientation via a
    rank-1 ones x dst matmul + is_equal, and per chunk in the [e, n]
    orientation via iota + is_equal.
  * Per 128-edge chunk: edge-MLP layer 1 = three accumulating matmuls
    (source term, target term via indicator x (x_tile @ W1_t + b1) table,
    edge_attr term), ReLU to bf16, then aggregation of the activations with
    an indicator matmul accumulated in PSUM ([f, n] orientation).  W_msg2 is
    applied after aggregation (segment_sum is linear), with deg(n)*b2 as a
    rank-1 matmul.
  * Update phase per tile (fp32): gated MLP + LayerNorm entirely on-chip.
"""

import numpy as np
import ml_dtypes

P = 128
N_NODES = 50000
N_EDGES = 640000
H = 128
EDGE_DIM = 3
NC = 8
PC = N_NODES // NC            # 6250 nodes per core
NT = (PC + P - 1) // P        # 49 tiles per core
LAST_BASE = PC - P            # 6122: base of (overlapping) last tile
HALF = 25000                  # int16 gather index split point

_kernel_cache = {}


# --------------------------------------------------------------------------
# Host-side preprocessing
# --------------------------------------------------------------------------

def _tile_base(t):
    return min(t * P, LAST_BASE) if t == NT - 1 else t * P


def _prep(x, edge_index, edge_attr):
    """Bucket/pad edges per (core, tile). Returns per-core arrays + schedule."""
    src = np.asarray(edge_index[0], dtype=np.int64)
    dst = np.asarray(edge_index[1], dtype=np.int64)
    ea = np.asarray(edge_attr, dtype=np.float32)

    per_core = []
    ka_ct = np.zeros((NC, NT), np.int64)
    kb_ct = np.zeros((NC, NT), np.int64)
    for c in range(NC):
        m = (dst // PC) == c
        d_l = dst[m] - c * PC
        s_l = src[m]
        ea_l = ea[m]
        tiles = []
        for t in range(NT):
            base = _tile_base(t)
            lo, hi = t * P, min((t + 1) * P, PC)
            tm = (d_l >= lo) & (d_l < hi)
            ss, ds, eat = s_l[tm], d_l[tm] - base, ea_l[tm]
            a_m = ss < HALF
            ia = np.where(a_m)[0]
            ib = np.where(~a_m)[0]
            ia = ia[np.argsort(ss[ia], kind="stable")]
            ib = ib[np.argsort(ss[ib], kind="stable")]
            ka_ct[c, t] = len(ia)
            kb_ct[c, t] = len(ib)
            tiles.append((ss, ds, eat, ia, ib))
        per_core.append(tiles)

    ca = np.ceil(ka_ct / P).astype(int).max(axis=0)  # per-tile A chunks
    cb = np.ceil(kb_ct / P).astype(int).max(axis=0)  # per-tile B chunks
    cpt = ca + cb
    cpt_max = int(cpt.max())
    ca_cols = max(int(ca.max()) * 8, 8)
    cb_cols = max(int(cb.max()) * 8, 8)

    maps = []
    for c in range(NC):
        idxA = np.zeros((NT, P, ca_cols), np.int16)
        idxB = np.zeros((NT, P, cb_cols), np.int16)
        ind2 = np.zeros((NT, P, 2 * cpt_max * P), ml_dtypes.bfloat16)
        g4 = (cpt_max + 3) // 4
        ea4 = np.zeros((NT, P, g4 * P), ml_dtypes.bfloat16)
        deg = np.zeros((NT, 1, P), ml_dtypes.bfloat16)
        for t in range(NT):
            ss, ds, eat, ia, ib = per_core[c][t]
            nA, nB = ca[t] * P, cb[t] * P
            sA = ss[ia].astype(np.int64)
            sB = ss[ib].astype(np.int64) - HALF
            iA = np.zeros(nA, np.int64)
            iA[: len(ia)] = sA
            iB = np.zeros(nB, np.int64)
            iB[: len(ib)] = sB
            # idx slot i lives at [i % 16, i // 16], replicated across the
            # eight 16-partition GPSIMD core groups
            blkA = iA.reshape(-1, 16).T.astype(np.int16)
            blkB = iB.reshape(-1, 16).T.astype(np.int16)
            for k in range(8):
                idxA[t, 16 * k : 16 * (k + 1), : ca[t] * 8] = blkA
                if cb[t]:
                    idxB[t, 16 * k : 16 * (k + 1), : cb[t] * 8] = blkB
            slots = np.concatenate([np.arange(len(ia)), nA + np.arange(len(ib))])
            order = np.concatenate([ia, ib])
            dso = ds[order]
            et = (ca[t] + cb[t]) * P
            dslot = np.full(et, -1, np.int64)
            dslot[slots] = dso
            onehot = dslot[:, None] == np.arange(P)[None, :]  # [et, P]
            oh3 = onehot.reshape(-1, P, P)  # [cpt, e, n]
            ind2[t][:, :et] = (
                oh3.transpose(1, 0, 2).reshape(P, et).astype(ml_dtypes.bfloat16)
            )
            ind2[t][:, cpt_max * P : cpt_max * P + et] = onehot.T.astype(
                ml_dtypes.bfloat16
            )
            eafull = np.zeros((et, EDGE_DIM), np.float32)
            eafull[slots] = eat[order]
            for ci in range((ca[t] + cb[t])):
                j = ci % 4
                ea4[t][32 * j : 32 * j + EDGE_DIM,
                       (ci // 4) * P : (ci // 4 + 1) * P] = (
                    eafull[ci * P : (ci + 1) * P].T.astype(ml_dtypes.bfloat16)
                )
            degf = np.zeros(P, np.float32)
            np.add.at(degf, dso, 1.0)
            deg[t, 0] = degf.astype(ml_dtypes.bfloat16)
        x_slice = np.zeros((NT * P, H), np.float32)
        for t in range(NT):
            b = _tile_base(t) + c * PC
            x_slice[t * P : (t + 1) * P] = x[b : b + P]
        maps.append(
            dict(idxA=idxA, idxB=idxB, ind2=ind2, ea4=ea4,
                 deg=deg, x_slice=x_slice)
        )
    return maps, tuple(int(v) for v in ca), tuple(int(v) for v in cb), cpt_max


# --------------------------------------------------------------------------
# Bass kernel builder
# --------------------------------------------------------------------------

def _build(ca, cb, cpt_max, ca_cols, cb_cols):
    import concourse.bacc as bacc
    import concourse.tile as tile
    from concourse import bass, mybir
    from concourse.masks import make_identity

    f32 = mybir.dt.float32
    bf16 = mybir.dt.bfloat16
    i16 = mybir.dt.int16
    Alu = mybir.AluOpType
    Act = mybir.ActivationFunctionType
    Axis = mybir.AxisListType

    nc = bacc.Bacc("TRN2", target_bir_lowering=False, debug=False, num_devices=NC)

    EMAX = cpt_max * P

    # ---- DRAM I/O ----
    x_bf = nc.dram_tensor("x_bf", [N_NODES, H], bf16, kind="ExternalInput")
    x_slice = nc.dram_tensor("x_slice", [NT * P, H], f32, kind="ExternalInput")
    idxA_d = nc.dram_tensor("idxA", [NT, P, ca_cols], i16, kind="ExternalInput")
    idxB_d = nc.dram_tensor("idxB", [NT, P, cb_cols], i16, kind="ExternalInput")
    G4 = (cpt_max + 3) // 4
    ind2_d = nc.dram_tensor("ind2", [NT, P, 2 * EMAX], bf16, kind="ExternalInput")
    ea4_d = nc.dram_tensor("ea4", [NT, P, G4 * P], bf16, kind="ExternalInput")
    deg_d = nc.dram_tensor("deg", [NT, 1, P], bf16, kind="ExternalInput")

    wnames = [
        ("w1sb", [H, H], bf16), ("w1t", [H, H], f32), ("w1e4", [P, H], bf16),
        ("b1b", [P, H], f32), ("w2", [H, H], f32), ("b2r", [1, H], f32),
        ("wgx", [H, H], f32), ("wga", [H, H], f32), ("bgr", [1, H], f32),
        ("wu1x", [H, H], f32), ("wu1a", [H, H], f32), ("bu1c", [H, 1], f32),
        ("wu2", [H, H], f32), ("bu2r", [1, H], f32),
        ("gammab", [P, H], f32), ("betab", [P, H], f32),
    ]
    wd = {n: nc.dram_tensor(n, s, d, kind="ExternalInput") for n, s, d in wnames}

    out_d = nc.dram_tensor("out", [NT * P, H], f32, kind="ExternalOutput")

    with tile.TileContext(nc) as tc:
        with (
            tc.tile_pool(name="const", bufs=1) as cpool,
            tc.tile_pool(name="sg", bufs=5) as sg_pool,
            tc.tile_pool(name="meta", bufs=3) as meta_pool,
            tc.tile_pool(name="work", bufs=4) as work_pool,
            tc.tile_pool(name="upd", bufs=2) as upd_pool,
            tc.tile_pool(name="ps_h1", bufs=2, space="PSUM") as ps_h1,
            tc.tile_pool(name="ps_agg", bufs=2, space="PSUM") as ps_agg,
            tc.tile_pool(name="ps_upd", bufs=2, space="PSUM") as ps_upd,
        ):
            # ---- constants ----
            ident = cpool.tile([P, P], f32)
            make_identity(nc, ident[:])
            ones_row = cpool.tile([1, P], f32)
            nc.vector.memset(ones_row[:], 1.0)
            eps_col = cpool.tile([P, 1], f32)
            nc.vector.memset(eps_col[:], 1e-5)
            W = {}
            for n, s, d in wnames:
                W[n] = cpool.tile(s, d, tag=n, name=f"w_{n}")
                nc.sync.dma_start(out=W[n][:], in_=wd[n][:])

            for t in range(NT):
                cpt = ca[t] + cb[t]
                et = cpt * P
                # ---- per-tile loads ----
                xt = upd_pool.tile([P, H], f32, tag="xt")
                nc.scalar.dma_start(out=xt[:], in_=x_slice[t * P : (t + 1) * P, :])
                ind2_sb = meta_pool.tile([P, 2 * EMAX], bf16, tag="ind2")
                nc.sync.dma_start(out=ind2_sb[:], in_=ind2_d[t, :, :])
                ea4_sb = meta_pool.tile([P, G4 * P], bf16, tag="ea4")
                nc.scalar.dma_start(out=ea4_sb[:], in_=ea4_d[t, :, :])
                deg_sb = meta_pool.tile([1, P], bf16, tag="deg")
                nc.scalar.dma_start(out=deg_sb[:], in_=deg_d[t, :, :])
                deg_f32 = meta_pool.tile([1, P], f32, tag="degf")
                nc.vector.tensor_copy(out=deg_f32[:], in_=deg_sb[:])

                # ---- transposed gathers of x[src].T -> [h, e] bf16 ----
                sgA = sg_pool.tile([P, 1, EMAX], bf16, tag="sgA")
                sgB = sg_pool.tile([P, 1, EMAX], bf16, tag="sgB")
                if ca[t]:
                    ia_sb = meta_pool.tile([P, ca_cols], i16, tag="ia")
                    nc.sync.dma_start(out=ia_sb[:], in_=idxA_d[t, :, :])
                    nc.gpsimd.dma_gather(
                        sgA[:, :, : ca[t] * P], x_bf[:HALF, :],
                        ia_sb[:, : ca[t] * 8], ca[t] * P, ca[t] * P, H,
                        transpose=True, single_packet=False,
                    )
                if cb[t]:
                    ib_sb = meta_pool.tile([P, cb_cols], i16, tag="ib")
                    nc.sync.dma_start(out=ib_sb[:], in_=idxB_d[t, :, :])
                    nc.gpsimd.dma_gather(
                        sgB[:, :, : cb[t] * P], x_bf[HALF:, :],
                        ib_sb[:, : cb[t] * 8], cb[t] * P, cb[t] * P, H,
                        transpose=True, single_packet=False,
                    )

                # ---- y = x_tile @ W1_t + b1 (target-term table, bf16) ----
                xT_ps = ps_upd.tile([P, P], f32, tag="u")
                nc.tensor.transpose(out=xT_ps[:], in_=xt[:], identity=ident[:])
                xT = upd_pool.tile([P, P], f32, tag="xT")
                nc.vector.tensor_copy(out=xT[:], in_=xT_ps[:])
                y_ps = ps_upd.tile([P, P], f32, tag="u")
                nc.tensor.matmul(
                    out=y_ps[:], lhsT=xT[:], rhs=W["w1t"][:], start=True, stop=True
                )
                y_bf = upd_pool.tile([P, P], bf16, tag="y")
                nc.vector.tensor_tensor(
                    out=y_bf[:], in0=y_ps[:], in1=W["b1b"][:], op=Alu.add
                )

                # ---- chunk loop (groups of 4) ----
                aggA_ps = ps_agg.tile([P, P], f32, tag="agg")
                for c0 in range(0, cpt, 4):
                    grp = list(range(c0, min(c0 + 4, cpt)))
                    h1s = {}
                    for ci in grp:
                        cs = slice(ci * P, (ci + 1) * P)
                        h1 = ps_h1.tile([P, P], f32, tag="h1", name=f"h1_{t}_{ci}")
                        h1s[ci] = h1
                        if ci < ca[t]:
                            src_lhsT = sgA[:, 0, ci * P : (ci + 1) * P]
                        else:
                            cj = ci - ca[t]
                            src_lhsT = sgB[:, 0, cj * P : (cj + 1) * P]
                        nc.tensor.matmul(
                            out=h1[:], lhsT=src_lhsT, rhs=W["w1sb"][:],
                            start=True, stop=False,
                        )
                        nc.tensor.matmul(
                            out=h1[:], lhsT=ind2_sb[:, EMAX + ci * P : EMAX + (ci + 1) * P],
                            rhs=y_bf[:], start=False, stop=False,
                        )
                    for ci in grp:
                        j = ci - c0
                        nc.tensor.matmul(
                            out=h1s[ci][:],
                            lhsT=ea4_sb[32 * j : 32 * j + EDGE_DIM,
                                        (ci // 4) * P : (ci // 4 + 1) * P],
                            rhs=W["w1e4"][32 * j : 32 * j + EDGE_DIM, :],
                            start=False, stop=True, tile_position=(32 * j, 0),
                        )
                    for ci in grp:
                        cs = slice(ci * P, (ci + 1) * P)
                        A_sb = work_pool.tile([P, P], bf16, tag="A",
                                              name=f"A_{t}_{ci}")
                        if ci % 2 == 0:
                            nc.scalar.activation(
                                out=A_sb[:], in_=h1s[ci][:], func=Act.Relu
                            )
                        else:
                            nc.vector.tensor_single_scalar(
                                out=A_sb[:], in_=h1s[ci][:], scalar=0.0,
                                op=Alu.max,
                            )
                        nc.tensor.matmul(
                            out=aggA_ps[:], lhsT=A_sb[:], rhs=ind2_sb[:, cs],
                            start=(ci == 0), stop=(ci == cpt - 1),
                        )

                # ---- agg2T = W2.T @ aggA_T + b2 deg^T  ([g, n]) ----
                aggA_sb = upd_pool.tile([P, P], f32, tag="aggA")
                nc.vector.tensor_copy(out=aggA_sb[:], in_=aggA_ps[:])
                agg2_ps = ps_upd.tile([P, P], f32, tag="u")
                nc.tensor.matmul(
                    out=agg2_ps[:], lhsT=W["w2"][:], rhs=aggA_sb[:],
                    start=True, stop=False,
                )
                nc.tensor.matmul(
                    out=agg2_ps[:], lhsT=W["b2r"][:], rhs=deg_f32[:],
                    start=False, stop=True,
                )
                agg2T = upd_pool.tile([P, P], f32, tag="agg2T")
                nc.vector.tensor_copy(out=agg2T[:], in_=agg2_ps[:])

                # ---- gate = sigmoid([x, agg] @ W_gate + b_gate) ----
                gate_ps = ps_upd.tile([P, P], f32, tag="u")
                nc.tensor.matmul(
                    out=gate_ps[:], lhsT=xT[:], rhs=W["wgx"][:],
                    start=True, stop=False,
                )
                nc.tensor.matmul(
                    out=gate_ps[:], lhsT=agg2T[:], rhs=W["wga"][:],
                    start=False, stop=False,
                )
                nc.tensor.matmul(
                    out=gate_ps[:], lhsT=ones_row[:], rhs=W["bgr"][:],
                    start=False, stop=True,
                )
                gate = upd_pool.tile([P, P], f32, tag="gate")
                nc.scalar.activation(out=gate[:], in_=gate_ps[:], func=Act.Sigmoid)

                # ---- update = relu([x, agg] @ W_upd1 + b_upd1) @ W_upd2 + b2
                u1_ps = ps_upd.tile([P, P], f32, tag="u")
                nc.tensor.matmul(
                    out=u1_ps[:], lhsT=W["wu1x"][:], rhs=xT[:],
                    start=True, stop=False,
                )
                nc.tensor.matmul(
                    out=u1_ps[:], lhsT=W["wu1a"][:], rhs=agg2T[:],
                    start=False, stop=True,
                )
                UT = upd_pool.tile([P, P], f32, tag="UT")
                nc.scalar.activation(
                    out=UT[:], in_=u1_ps[:], func=Act.Relu, bias=W["bu1c"][:, :1]
                )
                upd_ps = ps_upd.tile([P, P], f32, tag="u")
                nc.tensor.matmul(
                    out=upd_ps[:], lhsT=UT[:], rhs=W["wu2"][:],
                    start=True, stop=False,
                )
                nc.tensor.matmul(
                    out=upd_ps[:], lhsT=ones_row[:], rhs=W["bu2r"][:],
                    start=False, stop=True,
                )

                # ---- out0 = x + gate * (update - x); LayerNorm ----
                d1 = upd_pool.tile([P, P], f32, tag="d1")
                nc.vector.tensor_sub(out=d1[:], in0=upd_ps[:], in1=xt[:])
                d2 = upd_pool.tile([P, P], f32, tag="d2")
                nc.vector.tensor_mul(out=d2[:], in0=d1[:], in1=gate[:])
                out0 = upd_pool.tile([P, P], f32, tag="out0")
                nc.vector.tensor_add(out=out0[:], in0=d2[:], in1=xt[:])

                stat = upd_pool.tile([P, 4], f32, tag="stat")
                nc.vector.tensor_reduce(
                    out=stat[:, 0:1], in_=out0[:], axis=Axis.X, op=Alu.add
                )
                nc.vector.tensor_scalar(
                    out=stat[:, 1:2], in0=stat[:, 0:1],
                    scalar1=1.0 / H, scalar2=None, op0=Alu.mult,
                )
                cent = upd_pool.tile([P, P], f32, tag="cent")
                nc.vector.tensor_scalar(
                    out=cent[:], in0=out0[:],
                    scalar1=stat[:, 1:2], scalar2=None, op0=Alu.subtract,
                )
                sqd = upd_pool.tile([P, P], f32, tag="sqd")
                nc.scalar.activation(
                    out=sqd[:], in_=cent[:], func=Act.Square,
                    accum_out=stat[:, 2:3],
                )
                nc.scalar.activation(
                    out=stat[:, 3:4], in_=stat[:, 2:3], func=Act.Sqrt,
                    bias=eps_col[:, 0:1], scale=1.0 / H,
                )
                rinv = upd_pool.tile([P, 1], f32, tag="rinv")
                nc.vector.reciprocal(out=rinv[:], in_=stat[:, 3:4])
                normed = upd_pool.tile([P, P], f32, tag="normed")
                nc.vector.tensor_scalar(
                    out=normed[:], in0=cent[:],
                    scalar1=rinv[:, 0:1], scalar2=None, op0=Alu.mult,
                )
                g1 = upd_pool.tile([P, P], f32, tag="g1")
                nc.vector.tensor_mul(out=g1[:], in0=normed[:], in1=W["gammab"][:])
                outf = upd_pool.tile([P, P], f32, tag="outf")
                nc.vector.tensor_add(out=outf[:], in0=g1[:], in1=W["betab"][:])

                nc.sync.dma_start(
                    out=out_d[t * P : (t + 1) * P, :], in_=outf[:]
                )

    nc.compile()
    return nc


# --------------------------------------------------------------------------
# Public entry point
# --------------------------------------------------------------------------

def _weight_map(kw):
    b1 = kw["b_msg1"].astype(np.float32)
    bf = ml_dtypes.bfloat16
    w1e4 = np.zeros((P, H), np.float32)
    for j in range(4):
        w1e4[32 * j : 32 * j + EDGE_DIM] = kw["W_msg1"][2 * H :]
    return dict(
        w1sb=kw["W_msg1"][:H].astype(bf),
        w1t=kw["W_msg1"][H : 2 * H].astype(np.float32),
        w1e4=w1e4.astype(bf),
        b1b=np.tile(b1[None, :], (P, 1)),
        w2=kw["W_msg2"].astype(np.float32),
        b2r=kw["b_msg2"].astype(np.float32)[None, :],
        wgx=kw["W_gate"][:H].astype(np.float32),
        wga=kw["W_gate"][H:].astype(np.float32),
        bgr=kw["b_gate"].astype(np.float32)[None, :],
        wu1x=kw["W_upd1"][:H].astype(np.float32),
        wu1a=kw["W_upd1"][H:].astype(np.float32),
        bu1c=kw["b_upd1"].astype(np.float32)[:, None],
        wu2=kw["W_upd2"].astype(np.float32),
        bu2r=kw["b_upd2"].astype(np.float32)[None, :],
        gammab=np.tile(kw["ln_gamma"].astype(np.float32)[None, :], (P, 1)),
        betab=np.tile(kw["ln_beta"].astype(np.float32)[None, :], (P, 1)),
    )


def build_in_maps(**inputs):
    """Host prep: returns (nc-builder args, per-core input maps)."""
    x = np.asarray(inputs["x"], dtype=np.float32)
    maps, ca, cb, cpt_max = _prep(x, inputs["edge_index"], inputs["edge_attr"])
    wm = _weight_map(inputs)
    x_bf = x.astype(ml_dtypes.bfloat16)
    in_maps = []
    for c in range(NC):
        m = dict(maps[c])
        m["x_bf"] = x_bf
        m.update(wm)
        in_maps.append(m)
    ca_cols = max(max(ca) * 8, 8)
    cb_cols = max(max(cb) * 8, 8)
    return (ca, cb, cpt_max, ca_cols, cb_cols), in_maps


def get_kernel(build_args):
    if build_args not in _kernel_cache:
        _kernel_cache[build_args] = _build(*build_args)
    return _kernel_cache[build_args]


def assemble(results):
    """results: list of per-core out arrays [NT*P, H] -> [N_NODES, H]."""
    full = np.empty((N_NODES, H), np.float32)
    n_full = (NT - 1) * P          # 6144 rows from non-overlapping tiles
    off = n_full - LAST_BASE       # duplicated rows at start of last tile
    for c in range(NC):
        o = results[c]
        lo = c * PC
        full[lo : lo + n_full] = o[:n_full]
        full[lo + n_full : lo + PC] = o[n_full + off : n_full + off + (PC - n_full)]
    return full


def kernel(**inputs):
    import time
    from concourse.bass_utils import run_bass_kernel_spmd

    build_args, in_maps = build_in_maps(**inputs)
    nc = get_kernel(build_args)
    last_err = None
    for attempt in range(3):
        try:
            res = run_bass_kernel_spmd(nc, in_maps, list(range(NC)))
            outs = [res.results[c]["out"] for c in range(NC)]
            return assemble(outs)
        except Exception as e:  # transient device wedge: retry
            last_err = e
            time.sleep(2.0)
    raise last_err


if __name__ == "__main__":
    import reference

    inputs = {k: np.asarray(v) for k, v in reference.setup_inputs().items()}
    out = kernel(**inputs)
    exp = np.asarray(reference.reference(**reference.setup_inputs()))
    err = np.abs(out - exp).max() / (np.abs(exp).max() + 1e-12)
    print("Relative error:", err)
